# revision 1
# baseline (speedup 1.0000x reference)
"""Trainium2 Bass kernel for nn_CrossAttention (B=8, QL=KVL=2048, E=1024).

Sharding: data-parallel over batch — batch b runs on NeuronCore b.

Per-core dataflow (all fp32 storage, float32r matmuls):
  P1: qT = WqT.T@xqT+bq -> DRAM bounce; kT -> SBUF resident; v -> SBUF resident
  P2, per q-block: sT = kT.T@qT (PSUM), pT = exp(sT/32)*mask (SBUF),
      rowsums via PE ones-matmul, oT = v.T@pT, y = (oT.T@WoT)/rowsum + bo.
  Everything stays in the [feature-on-partition] layout so no on-chip
  transposes are needed; the host pre-transposes activations and weights.
"""

import os
import sys

import numpy as np

for _p in ("/opt/trn_rl_repo", "/opt/pypackages"):
    if _p not in sys.path and os.path.isdir(_p):
        sys.path.append(_p)

import concourse.bass as bass
import concourse.mybir as mybir
import concourse.tile as tile
from concourse.bass_utils import run_bass_kernel_spmd
from concourse.vector_clock import ScopedClock

F32 = mybir.dt.float32
F32R = mybir.dt.float32r
AF = mybir.ActivationFunctionType
ALU = mybir.AluOpType


def _ensure_ntff_hook():
    """The agent image's antenv lacks axon_hooks, so the boot-time NTFF
    profile hook registration silently degraded. Recreate the module and
    register the ctypes-based hook against libaxon_pjrt.so so trace=True
    runs produce per-core NTFF profiles (HW exec time)."""
    try:
        from antenv.axon_hooks import get_axon_ntff_profile_hook  # noqa: F401

        return
    except ImportError:
        pass
    import contextlib
    import ctypes
    import types

    import antenv

    mod = types.ModuleType("antenv.axon_hooks")
    mod._hook = None

    def set_axon_ntff_profile_hook(h):
        mod._hook = h

    def get_axon_ntff_profile_hook():
        return mod._hook

    mod.set_axon_ntff_profile_hook = set_axon_ntff_profile_hook
    mod.get_axon_ntff_profile_hook = get_axon_ntff_profile_hook
    sys.modules["antenv.axon_hooks"] = mod
    antenv.axon_hooks = mod

    so_path = "/opt/axon/libaxon_pjrt.so"
    if not os.path.exists(so_path):
        return
    lib = ctypes.CDLL(so_path)
    if not hasattr(lib, "axon_start_nrt_profile"):
        return
    lib.axon_start_nrt_profile.argtypes = [
        ctypes.POINTER(ctypes.c_int64),
        ctypes.c_size_t,
    ]
    lib.axon_start_nrt_profile.restype = ctypes.c_int64
    lib.axon_stop_nrt_profile.argtypes = [ctypes.c_char_p]
    lib.axon_stop_nrt_profile.restype = ctypes.c_int64

    @contextlib.contextmanager
    def _hook(output_dir, device_ids):
        import jax

        jax.devices()
        if device_ids:
            ids = (ctypes.c_int64 * len(device_ids))(*device_ids)
            rc = lib.axon_start_nrt_profile(ids, len(device_ids))
        else:
            rc = lib.axon_start_nrt_profile(None, 0)
        if rc != 0:
            raise RuntimeError(f"axon_start_nrt_profile rc={rc}")
        try:
            yield
        finally:
            n = lib.axon_stop_nrt_profile(str(output_dir).encode())
            print(f"ntff profile: {n} file(s) written to {output_dir}")

    set_axon_ntff_profile_hook(_hook)


_ensure_ntff_hook()

B, QL, KVL, E = 8, 2048, 2048, 1024
P = 128
EC = E // P          # 8 feature chunks
SCALE = 1.0 / 32.0   # 1/sqrt(E)


class _TC(tile.TileContext):
    """TileContext whose final drain never carries >1 sync wait.

    The walrus build in this container rejects instructions with more than
    one sync-wait command; spread the drain's waits across single-wait NOPs.
    """

    def _drain_and_barrier(self, tick_clock, wait_clock):
        nc = self.nc
        probe = nc.sync.nop(nofuse=True, hint="drain_wait_probe")
        wait_clock.add_sem_waits(
            probe.ins, ScopedClock({None: tick_clock.global_clock})
        )
        si = probe.ins.sync_info
        waits = list(si.on_wait) if si is not None else []
        if len(waits) > 1:
            probe.ins.sync_info = mybir.SyncInfo(
                on_wait=waits[:1], on_update=list(si.on_update)
            )
            for w in waits[1:]:
                extra = nc.sync.nop(nofuse=True, hint="drain_wait_spill")
                extra.ins.sync_info = mybir.SyncInfo(on_wait=[w], on_update=[])
        nc.sync.drain()
        nc.all_engine_barrier()
        assert self.sems is not None
        popped = nc._tile_sem_poison_stack.pop()
        assert popped is self._sem_poison
        nc.clear_and_free_semaphores(list(self.sems.allocated().values()))
        nc.all_engine_barrier()


def _split_multi_waits(nc):
    """Walrus here allows only one sync-wait per instruction; hoist extras
    onto same-engine NOPs inserted immediately before."""
    idx = 0
    for fn in nc.m.functions:
        for blk in fn.blocks:
            out = []
            changed = False
            for inst in blk.instructions:
                si = inst.sync_info
                if si is not None and len(si.on_wait) > 1:
                    changed = True
                    waits = list(si.on_wait)
                    for w in waits[:-1]:
                        nop = mybir.InstNoOp(name=f"I-waitsplit-{idx}")
                        idx += 1
                        nop.engine = inst.engine
                        nop.sync_info = mybir.SyncInfo(on_wait=[w], on_update=[])
                        out.append(nop)
                    inst.sync_info = mybir.SyncInfo(
                        on_wait=[waits[-1]], on_update=list(si.on_update)
                    )
                out.append(inst)
            if changed:
                blk.instructions = out



class _WHalves:
    """Two [P, EC, 512] half-tiles presented as one [P, EC, 1024] tensor."""

    def __init__(self, lo, hi):
        self._h = (lo, hi)

    def __getitem__(self, key):
        p, ec, o = key
        if isinstance(o, slice):
            start, stop = o.start or 0, o.stop
            if stop <= 512:
                return self._h[0][p, ec, start:stop]
            assert start >= 512
            return self._h[1][p, ec, start - 512 : stop - 512]
        raise TypeError(o)


def _load_w_halves(nc, wpool, w_v):
    halves = []
    for i in range(2):
        t = wpool.tile([P, EC, 512], F32R, tag=f"wh{i}")
        nc.sync.dma_start(out=t[:], in_=w_v[:, :, i * 512 : (i + 1) * 512])
        halves.append(t)
    return _WHalves(*halves)


def build_nc(ql=QL, kvl=KVL, qb=256, lb=512):
    """Build the single-core Bass program (same program runs on all 8 cores)."""
    kc = kvl // P        # kv chunks of 128
    nqb = ql // qb       # q blocks in phase 2
    qq_n = qb // P       # 128-row subblocks per q block
    eo2_n = E // 512

    nc = bass.Bass("TRN2", target_bir_lowering=False, debug=False)

    # activations arrive host-blocked: [block, partition, e-chunk, block-col]
    # so each x-block DMA reads 16KB contiguous per partition
    xq = nc.dram_tensor(
        "xq_blk", [ql // lb, P, EC, lb], F32R, kind="ExternalInput"
    ).ap()
    xkv = nc.dram_tensor(
        "xkv_blk", [kvl // lb, P, EC, lb], F32R, kind="ExternalInput"
    ).ap()
    maskb = nc.dram_tensor(
        "maskblk", [kc, nqb, P, qb], F32, kind="ExternalInput"
    ).ap()
    wq = nc.dram_tensor("wqT", [E, E], F32R, kind="ExternalInput").ap()
    wk = nc.dram_tensor("wkT", [E, E], F32R, kind="ExternalInput").ap()
    wv = nc.dram_tensor("wvT", [E, E], F32R, kind="ExternalInput").ap()
    wo = nc.dram_tensor("woT", [E, E], F32R, kind="ExternalInput").ap()
    bq = nc.dram_tensor("bq_pp", [P, EC], F32, kind="ExternalInput").ap()
    bk = nc.dram_tensor("bk_pp", [P, EC], F32, kind="ExternalInput").ap()
    bvr = nc.dram_tensor("bv_rep", [P, E], F32, kind="ExternalInput").ap()
    bor = nc.dram_tensor("bo_rep", [P, E], F32, kind="ExternalInput").ap()
    ones_in = nc.dram_tensor("ones", [P, 4], F32R, kind="ExternalInput").ap()
    y = nc.dram_tensor("y", [ql, E], F32, kind="ExternalOutput").ap()

    wq_v = wq.rearrange("(ec p) o -> p ec o", p=P)
    wk_v = wk.rearrange("(ec p) o -> p ec o", p=P)
    wv_v = wv.rearrange("(ec p) o -> p ec o", p=P)
    wo_v = wo.rearrange("(ec p) o -> p ec o", p=P)

    with _TC(nc) as tc:
        with (
            tc.tile_pool(name="persist", bufs=1) as persist,
            tc.tile_pool(name="consts", bufs=1) as consts,
            tc.tile_pool(name="wpool", bufs=1) as wpool,
            tc.tile_pool(name="dramp", bufs=1, space="DRAM") as dramp,
        ):
            kt = persist.tile([P, EC, kvl], F32R, tag="kt")
            vv = persist.tile([P, kc, E], F32R, tag="vv")
            qtd = dramp.tile([ql // qb, P, EC, qb], F32R, tag="qtd")

            bq_sb = consts.tile([P, EC], F32, tag="bq")
            bk_sb = consts.tile([P, EC], F32, tag="bk")
            bvr_sb = consts.tile([P, E], F32, tag="bvr")
            ones = consts.tile([P, 4], F32R, tag="ones")
            nc.sync.dma_start(out=bq_sb[:], in_=bq)
            nc.sync.dma_start(out=ones[:], in_=ones_in)

            # ---------------- Phase 1: projections ----------------
            with (
                tc.tile_pool(name="p1x", bufs=2) as xpool,
                tc.tile_pool(name="p1qts", bufs=4) as qtsp,
                tc.tile_pool(name="p1ps", bufs=6, space="PSUM") as pp1,
            ):
                # qT = (WqT.T @ xqT) + bq  -> DRAM bounce
                # weights load as two 512-wide halves in separate slots so
                # the next stage's halves stream in behind this stage's use
                xblk = xpool.tile([P, EC, lb], F32R, tag="x")
                nc.sync.dma_start(out=xblk[:], in_=xq[0])
                wq_sb = _load_w_halves(nc, wpool, wq_v)
                for qlb in range(ql // lb):
                    if qlb > 0:
                        xblk = xpool.tile([P, EC, lb], F32R, tag="x")
                        nc.sync.dma_start(out=xblk[:], in_=xq[qlb])
                    for eo in range(EC):
                        ps = pp1.tile([P, lb], F32, tag="ps")
                        for ei in range(EC):
                            nc.tensor.matmul(
                                ps[:],
                                lhsT=(wq_sb[:, ei, eo * P : (eo + 1) * P]),
                                rhs=(xblk[:, ei, :]),
                                start=(ei == 0),
                                stop=(ei == EC - 1),
                            )
                        qt_t = qtsp.tile([P, lb], F32R, tag="qts")
                        nc.scalar.activation(
                            qt_t[:], ps[:], AF.Identity, bias=bq_sb[:, eo : eo + 1]
                        )
                        for j in range(lb // qb):
                            nc.sync.dma_start(
                                out=qtd[qlb * (lb // qb) + j, :, eo, :],
                                in_=qt_t[:, j * qb : (j + 1) * qb],
                            )

                # kT = (WkT.T @ xkvT) + bk -> SBUF resident
                nc.sync.dma_start(out=bk_sb[:], in_=bk)
                xblk = xpool.tile([P, EC, lb], F32R, tag="x")
                nc.sync.dma_start(out=xblk[:], in_=xkv[0])
                wk_sb = _load_w_halves(nc, wpool, wk_v)
                for kvb in range(kvl // lb):
                    if kvb > 0:
                        xblk = xpool.tile([P, EC, lb], F32R, tag="x")
                        nc.sync.dma_start(out=xblk[:], in_=xkv[kvb])
                    for eo in range(EC):
                        ps = pp1.tile([P, lb], F32, tag="ps")
                        for ei in range(EC):
                            nc.tensor.matmul(
                                ps[:],
                                lhsT=(wk_sb[:, ei, eo * P : (eo + 1) * P]),
                                rhs=(xblk[:, ei, :]),
                                start=(ei == 0),
                                stop=(ei == EC - 1),
                            )
                        nc.scalar.activation(
                            kt[:, eo, kvb * lb : (kvb + 1) * lb],
                            ps[:],
                            AF.Identity,
                            bias=bk_sb[:, eo : eo + 1],
                        )

                # v = (xkvT.T @ WvT) + bv -> SBUF resident, [kv, E] layout
                # first xblk DMA is triggered before the weight loads so it
                # is not stuck behind 4MB of wv in the DMA queue FIFOs
                nc.sync.dma_start(out=bvr_sb[:], in_=bvr)
                xblk = xpool.tile([P, EC, lb], F32R, tag="x")
                nc.sync.dma_start(out=xblk[:], in_=xkv[0])
                wv_sb = _load_w_halves(nc, wpool, wv_v)
                for kvb in range(kvl // lb):
                    if kvb > 0:
                        xblk = xpool.tile([P, EC, lb], F32R, tag="x")
                        nc.sync.dma_start(out=xblk[:], in_=xkv[kvb])
                    for k2 in range(lb // P):
                        kvc = kvb * (lb // P) + k2
                        for eo2 in range(eo2_n):
                            ps = pp1.tile([P, 512], F32, tag="ps")
                            for ei in range(EC):
                                nc.tensor.matmul(
                                    ps[:],
                                    lhsT=(xblk[:, ei, k2 * P : (k2 + 1) * P]),
                                    rhs=(wv_sb[:, ei, eo2 * 512 : (eo2 + 1) * 512]),
                                    start=(ei == 0),
                                    stop=(ei == EC - 1),
                                )
                            nc.vector.tensor_tensor(
                                vv[:, kvc, eo2 * 512 : (eo2 + 1) * 512],
                                ps[:],
                                bvr_sb[:, eo2 * 512 : (eo2 + 1) * 512],
                                ALU.add,
                            )

            # ---------------- Phase 2: attention + output ----------------
            wo_sb = None
            with (
                tc.tile_pool(name="p2pt", bufs=1) as ptp,
                tc.tile_pool(name="p2o", bufs=1) as osp,
                tc.tile_pool(name="p2bor", bufs=1) as borp,
                tc.tile_pool(name="p2small", bufs=3) as smallp,
                tc.tile_pool(name="p2out", bufs=2) as outp,
                tc.tile_pool(name="p2qtb", bufs=1) as qtbp,
                tc.tile_pool(name="p2ps_s", bufs=3, space="PSUM") as pss,
                tc.tile_pool(name="p2ps_rs", bufs=1, space="PSUM") as psr,
                tc.tile_pool(name="p2ps_o", bufs=2, space="PSUM") as pso,
                tc.tile_pool(name="p2ps_f", bufs=2, space="PSUM") as psf,
            ):
                bor_sb = borp.tile([P, E], F32, tag="bor")
                nc.sync.dma_start(out=bor_sb[:], in_=bor)
                for iqb in range(nqb):
                    qtb = qtbp.tile([P, EC, qb], F32R, tag="qtb")
                    nc.sync.dma_start(out=qtb[:], in_=qtd[iqb])
                    pt = ptp.tile([P, kc, qb], F32R, tag="pt")
                    for c in range(kc):
                        ps = pss.tile([P, qb], F32, tag="s")
                        for e in range(EC):
                            nc.tensor.matmul(
                                ps[:],
                                lhsT=(kt[:, e, c * P : (c + 1) * P]),
                                rhs=(qtb[:, e, :]),
                                start=(e == 0),
                                stop=(e == EC - 1),
                            )
                        nc.scalar.activation(
                            pt[:, c, :], ps[:], AF.Exp, scale=SCALE
                        )
                        mt = smallp.tile([P, qb], F32, tag="mask")
                        nc.sync.dma_start(out=mt[:], in_=maskb[c, iqb, :, :])
                        nc.vector.tensor_tensor(
                            pt[:, c, :], pt[:, c, :], mt[:], ALU.mult
                        )

                    recip = smallp.tile([P, qq_n], F32, tag="recip")
                    for qq in range(qq_n):
                        rs = psr.tile([P, 4], F32, tag="rs")
                        for c in range(kc):
                            nc.tensor.matmul(
                                rs[:],
                                lhsT=(pt[:, c, qq * P : (qq + 1) * P]),
                                rhs=(ones[:]),
                                start=(c == 0),
                                stop=(c == kc - 1),
                            )
                        nc.vector.reciprocal(recip[:, qq : qq + 1], rs[:, 0:1])

                    if wo_sb is None:
                        wo_sb = _load_w_halves(nc, wpool, wo_v)
                    osb = osp.tile([P, EC, qb], F32R, tag="o")
                    for m in range(EC):
                        po = pso.tile([P, qb], F32, tag="o")
                        for c in range(kc):
                            nc.tensor.matmul(
                                po[:],
                                lhsT=(vv[:, c, m * P : (m + 1) * P]),
                                rhs=(pt[:, c, :]),
                                start=(c == 0),
                                stop=(c == kc - 1),
                            )
                        nc.scalar.activation(osb[:, m, :], po[:], AF.Copy)

                    for eo2 in range(eo2_n):
                        for qq in range(qq_n):
                            pf = psf.tile([P, 512], F32, tag="f")
                            for m in range(EC):
                                nc.tensor.matmul(
                                    pf[:],
                                    lhsT=(osb[:, m, qq * P : (qq + 1) * P]),
                                    rhs=(wo_sb[:, m, eo2 * 512 : (eo2 + 1) * 512]),
                                    start=(m == 0),
                                    stop=(m == EC - 1),
                                )
                            ot = outp.tile([P, 512], F32, tag="out")
                            nc.vector.scalar_tensor_tensor(
                                ot[:],
                                pf[:],
                                recip[:, qq : qq + 1],
                                bor_sb[:, eo2 * 512 : (eo2 + 1) * 512],
                                ALU.mult,
                                ALU.add,
                            )
                            nc.sync.dma_start(
                                out=y[
                                    iqb * qb + qq * P : iqb * qb + (qq + 1) * P,
                                    eo2 * 512 : (eo2 + 1) * 512,
                                ],
                                in_=ot[:],
                            )

    _split_multi_waits(nc)
    return nc


_NC_CACHE = {}


def _get_nc(ql=QL, kvl=KVL):
    key = (ql, kvl)
    if key not in _NC_CACHE:
        _NC_CACHE[key] = build_nc(ql=ql, kvl=kvl)
    return _NC_CACHE[key]


def _round_fp32r(a):
    """Round-to-nearest fp32->fp32r (11 explicit mantissa bits), matching
    walrus's fp32_to_fp32r. The PE consumes fp32r operands; pre-rounding on
    the host keeps DMA-loaded data consistent with the declared dtype."""
    b = np.ascontiguousarray(a, dtype=np.float32)
    u = b.view(np.uint32)
    u += 0x800
    u &= 0xFFFFF000
    return b


def _host_prep(query, key_value, attention_mask, Wq, bq, Wk, bk, Wv, bv, Wo, bo):
    """Build the 8 per-core input maps (numpy only)."""
    b, ql, e = query.shape
    kvl = key_value.shape[1]
    qb = 256
    kc, nqb = kvl // P, ql // qb

    f32 = np.float32
    shared = {
        "wqT": _round_fp32r(Wq.T),
        "wkT": _round_fp32r(Wk.T),
        "wvT": _round_fp32r(Wv.T),
        "woT": _round_fp32r(Wo.T),
        "bq_pp": np.ascontiguousarray(bq.reshape(EC, P).T, dtype=f32),
        "bk_pp": np.ascontiguousarray(bk.reshape(EC, P).T, dtype=f32),
        "bv_rep": np.ascontiguousarray(np.broadcast_to(bv, (P, e)), dtype=f32),
        "bo_rep": np.ascontiguousarray(np.broadcast_to(bo, (P, e)), dtype=f32),
        "ones": np.ones((P, 4), dtype=f32),
    }
    in_maps = []
    for i in range(b):
        m = attention_mask[i].T.astype(f32)  # [kv, q]
        mblk = np.ascontiguousarray(
            m.reshape(kc, P, nqb, qb).transpose(0, 2, 1, 3)
        )
        lb = 512
        xqb = _round_fp32r(
            query[i].T.reshape(EC, P, ql // lb, lb).transpose(2, 1, 0, 3)
        )
        xkvb = _round_fp32r(
            key_value[i].T.reshape(EC, P, kvl // lb, lb).transpose(2, 1, 0, 3)
        )
        in_maps.append(
            dict(shared, xq_blk=xqb, xkv_blk=xkvb, maskblk=mblk)
        )
    return in_maps


def run(inputs, trace=False):
    """Run on 8 cores; returns (output [B, QL, E], BassKernelResults)."""
    nc = _get_nc()
    in_maps = _host_prep(**inputs)
    res = run_bass_kernel_spmd(
        nc, in_maps, list(range(8)), trace=trace, trace_cores=[0]
    )
    out = np.stack([res.results[i]["y"] for i in range(8)], axis=0)
    return out, res


def kernel(**inputs):
    out, _ = run(inputs, trace=False)
    return out



# revision 4
# speedup vs baseline: 1.3184x; 1.3184x over previous
"""Trainium2 Bass kernel for nn_CrossAttention (B=8, QL=KVL=2048, E=1024).

Sharding: data-parallel over batch — batch b runs on NeuronCore b.

Per-core dataflow, all-bf16 operands (fp32 PSUM accumulation):
  P1: qT, kT -> SBUF-resident bf16 [feat-part, seq]; v -> SBUF-resident
      bf16 [seq-part, feat], K and V computed in one pass over xkv.
  P2, per 512-wide q block: sT = kT.T@qT (PSUM f32), pT = exp(sT/32)
      (bf16) * mask, row sums accumulated on DVE then one tiny matmul per
      128-q chunk, oT = v.T@pT, y = (oT.T@WoT)*recip + bo.
  No DRAM bounce: everything stays resident; the only phase-2 DMA is the
  (prefetched) mask stream in and y out.  Two HWDGE queues (sync + ACT)
  split weight/x loads so the PE primes in ~3us.
"""

import os
import sys

import numpy as np

for _p in ("/opt/trn_rl_repo", "/opt/pypackages"):
    if _p not in sys.path and os.path.isdir(_p):
        sys.path.append(_p)

import concourse.bass as bass
import concourse.mybir as mybir
import concourse.tile as tile
from concourse.bass_utils import run_bass_kernel_spmd
from concourse.vector_clock import ScopedClock

F32 = mybir.dt.float32
F32R = mybir.dt.float32r
BF16 = mybir.dt.bfloat16
AF = mybir.ActivationFunctionType
ALU = mybir.AluOpType


def _ensure_ntff_hook():
    """The agent image's antenv lacks axon_hooks, so the boot-time NTFF
    profile hook registration silently degraded. Recreate the module and
    register the ctypes-based hook against libaxon_pjrt.so so trace=True
    runs produce per-core NTFF profiles (HW exec time)."""
    try:
        from antenv.axon_hooks import get_axon_ntff_profile_hook  # noqa: F401

        return
    except ImportError:
        pass
    import contextlib
    import ctypes
    import types

    import antenv

    mod = types.ModuleType("antenv.axon_hooks")
    mod._hook = None

    def set_axon_ntff_profile_hook(h):
        mod._hook = h

    def get_axon_ntff_profile_hook():
        return mod._hook

    mod.set_axon_ntff_profile_hook = set_axon_ntff_profile_hook
    mod.get_axon_ntff_profile_hook = get_axon_ntff_profile_hook
    sys.modules["antenv.axon_hooks"] = mod
    antenv.axon_hooks = mod

    so_path = "/opt/axon/libaxon_pjrt.so"
    if not os.path.exists(so_path):
        return
    lib = ctypes.CDLL(so_path)
    if not hasattr(lib, "axon_start_nrt_profile"):
        return
    lib.axon_start_nrt_profile.argtypes = [
        ctypes.POINTER(ctypes.c_int64),
        ctypes.c_size_t,
    ]
    lib.axon_start_nrt_profile.restype = ctypes.c_int64
    lib.axon_stop_nrt_profile.argtypes = [ctypes.c_char_p]
    lib.axon_stop_nrt_profile.restype = ctypes.c_int64

    @contextlib.contextmanager
    def _hook(output_dir, device_ids):
        import jax

        jax.devices()
        if device_ids:
            ids = (ctypes.c_int64 * len(device_ids))(*device_ids)
            rc = lib.axon_start_nrt_profile(ids, len(device_ids))
        else:
            rc = lib.axon_start_nrt_profile(None, 0)
        if rc != 0:
            raise RuntimeError(f"axon_start_nrt_profile rc={rc}")
        try:
            yield
        finally:
            n = lib.axon_stop_nrt_profile(str(output_dir).encode())
            print(f"ntff profile: {n} file(s) written to {output_dir}")

    set_axon_ntff_profile_hook(_hook)


_ensure_ntff_hook()

B, QL, KVL, E = 8, 2048, 2048, 1024
P = 128
EC = E // P          # 8 feature chunks
SCALE = 1.0 / 32.0   # 1/sqrt(E)
QB = 512             # q block (moving-operand width) in phase 2
LB = 512             # x block in phase 1


class _TC(tile.TileContext):
    """TileContext whose final drain never carries >1 sync wait.

    The walrus build in this container rejects instructions with more than
    one sync-wait command; spread the drain's waits across single-wait NOPs.
    """

    def _drain_and_barrier(self, tick_clock, wait_clock):
        nc = self.nc
        probe = nc.sync.nop(nofuse=True, hint="drain_wait_probe")
        wait_clock.add_sem_waits(
            probe.ins, ScopedClock({None: tick_clock.global_clock})
        )
        si = probe.ins.sync_info
        waits = list(si.on_wait) if si is not None else []
        if len(waits) > 1:
            probe.ins.sync_info = mybir.SyncInfo(
                on_wait=waits[:1], on_update=list(si.on_update)
            )
            for w in waits[1:]:
                extra = nc.sync.nop(nofuse=True, hint="drain_wait_spill")
                extra.ins.sync_info = mybir.SyncInfo(on_wait=[w], on_update=[])
        nc.sync.drain()
        nc.all_engine_barrier()
        assert self.sems is not None
        popped = nc._tile_sem_poison_stack.pop()
        assert popped is self._sem_poison
        nc.clear_and_free_semaphores(list(self.sems.allocated().values()))
        nc.all_engine_barrier()


def _split_multi_waits(nc):
    """Walrus here allows only one sync-wait per instruction; hoist extras
    onto same-engine NOPs inserted immediately before."""
    idx = 0
    for fn in nc.m.functions:
        for blk in fn.blocks:
            out = []
            changed = False
            for inst in blk.instructions:
                si = inst.sync_info
                if si is not None and len(si.on_wait) > 1:
                    changed = True
                    waits = list(si.on_wait)
                    for w in waits[:-1]:
                        nop = mybir.InstNoOp(name=f"I-waitsplit-{idx}")
                        idx += 1
                        nop.engine = inst.engine
                        nop.sync_info = mybir.SyncInfo(on_wait=[w], on_update=[])
                        out.append(nop)
                    inst.sync_info = mybir.SyncInfo(
                        on_wait=[waits[-1]], on_update=list(si.on_update)
                    )
                out.append(inst)
            if changed:
                blk.instructions = out


class _WHalves:
    """Two [P, EC, 512] half-tiles presented as one [P, EC, 1024] tensor."""

    def __init__(self, lo, hi):
        self._h = (lo, hi)

    def __getitem__(self, key):
        p, ec, o = key
        if isinstance(o, slice):
            start, stop = o.start or 0, o.stop
            if stop <= 512:
                return self._h[0][p, ec, start:stop]
            assert start >= 512
            return self._h[1][p, ec, start - 512 : stop - 512]
        raise TypeError(o)


def _load_w_halves(nc, wpool, w_v, eng):
    halves = []
    for i in range(2):
        t = wpool.tile([P, EC, 512], BF16, tag=f"wh{i}")
        eng.dma_start(out=t[:], in_=w_v[:, :, i * 512 : (i + 1) * 512])
        halves.append(t)
    return _WHalves(*halves)


def build_nc(ql=QL, kvl=KVL):
    """Build the single-core Bass program (same program runs on all 8 cores)."""
    kc = kvl // P        # kv chunks of 128
    nqb = ql // QB       # q blocks in phase 2
    qq_n = QB // P       # 128-row subblocks per q block
    eo2_n = E // 512

    nc = bass.Bass("TRN2", target_bir_lowering=False, debug=False)

    # activations arrive host-blocked: [block, partition, e-chunk, block-col]
    xq = nc.dram_tensor(
        "xq_blk", [ql // LB, P, EC, LB], BF16, kind="ExternalInput"
    ).ap()
    xkv = nc.dram_tensor(
        "xkv_blk", [kvl // LB, P, EC, LB], BF16, kind="ExternalInput"
    ).ap()
    maskb = nc.dram_tensor(
        "maskblk", [nqb, P, kc, QB], BF16, kind="ExternalInput"
    ).ap()
    wq = nc.dram_tensor("wqT", [E, E], BF16, kind="ExternalInput").ap()
    wk = nc.dram_tensor("wkT", [E, E], BF16, kind="ExternalInput").ap()
    wv = nc.dram_tensor("wvT", [E, E], BF16, kind="ExternalInput").ap()
    wo = nc.dram_tensor("woT", [E, E], BF16, kind="ExternalInput").ap()
    bq = nc.dram_tensor("bq_pp", [P, EC], F32, kind="ExternalInput").ap()
    bk = nc.dram_tensor("bk_pp", [P, EC], F32, kind="ExternalInput").ap()
    bvr = nc.dram_tensor("bv_rep", [P, E], F32, kind="ExternalInput").ap()
    bor = nc.dram_tensor("bo_rep", [P, E], F32, kind="ExternalInput").ap()
    ones_in = nc.dram_tensor("ones", [P, 4], F32R, kind="ExternalInput").ap()
    y = nc.dram_tensor("y", [ql, E], F32, kind="ExternalOutput").ap()

    wq_v = wq.rearrange("(ec p) o -> p ec o", p=P)
    wk_v = wk.rearrange("(ec p) o -> p ec o", p=P)
    wv_v = wv.rearrange("(ec p) o -> p ec o", p=P)
    wo_v = wo.rearrange("(ec p) o -> p ec o", p=P)

    with _TC(nc) as tc:
        with (
            tc.tile_pool(name="persist", bufs=1) as persist,
            tc.tile_pool(name="consts", bufs=1) as consts,
            tc.tile_pool(name="wpool", bufs=2) as wpool,
            tc.tile_pool(name="maskp", bufs=2) as maskp,
        ):
            kt = persist.tile([P, EC, kvl], BF16, tag="kt")
            qt = persist.tile([P, EC, ql], BF16, tag="qt")
            vv = persist.tile([P, kc, E], BF16, tag="vv")

            bq_sb = consts.tile([P, EC], F32, tag="bq")
            bk_sb = consts.tile([P, EC], F32, tag="bk")
            bvr_sb = consts.tile([P, E], F32, tag="bvr")
            bor_sb = consts.tile([P, E], F32, tag="bor")
            ones = consts.tile([P, 4], F32R, tag="ones")

            # sync queue: biases + weights (phase-1 critical path);
            # scalar(ACT) queue: x blocks.  The two HWDGE rings drain in
            # parallel so the first matmul starts after ~1MB, not ~3MB.
            nc.sync.dma_start(out=bq_sb[:], in_=bq)

            # ---------------- Phase 1: projections ----------------
            with (
                tc.tile_pool(name="p1x", bufs=2) as xpool,
                tc.tile_pool(name="p1ps", bufs=3, space="PSUM") as pp1,
            ):
                xblk = xpool.tile([P, EC, LB], BF16, tag="x")
                nc.scalar.dma_start(out=xblk[:], in_=xq[0])
                wq_sb = _load_w_halves(nc, wpool, wq_v, nc.sync)
                nc.sync.dma_start(out=bk_sb[:], in_=bk)

                # qT = (WqT.T @ xqT) + bq -> SBUF resident bf16
                for qlb in range(ql // LB):
                    if qlb > 0:
                        xblk = xpool.tile([P, EC, LB], BF16, tag="x")
                        nc.scalar.dma_start(out=xblk[:], in_=xq[qlb])
                    for eo in range(EC):
                        ps = pp1.tile([P, LB], F32, tag="ps")
                        for ei in range(EC):
                            nc.tensor.matmul(
                                ps[:],
                                lhsT=(wq_sb[:, ei, eo * P : (eo + 1) * P]),
                                rhs=(xblk[:, ei, :]),
                                start=(ei == 0),
                                stop=(ei == EC - 1),
                            )
                        nc.scalar.activation(
                            qt[:, eo, qlb * LB : (qlb + 1) * LB],
                            ps[:],
                            AF.Identity,
                            bias=bq_sb[:, eo : eo + 1],
                        )

                # kT and v in one pass over xkv.
                xblk = xpool.tile([P, EC, LB], BF16, tag="x")
                nc.scalar.dma_start(out=xblk[:], in_=xkv[0])
                wk_sb = _load_w_halves(nc, wpool, wk_v, nc.sync)
                wv_sb = _load_w_halves(nc, wpool, wv_v, nc.sync)
                nc.sync.dma_start(out=bvr_sb[:], in_=bvr)
                nc.sync.dma_start(out=ones[:], in_=ones_in)
                for kvb in range(kvl // LB):
                    if kvb > 0:
                        xblk = xpool.tile([P, EC, LB], BF16, tag="x")
                        nc.scalar.dma_start(out=xblk[:], in_=xkv[kvb])
                    # kT = (WkT.T @ xkvT) + bk -> SBUF resident bf16
                    for eo in range(EC):
                        ps = pp1.tile([P, LB], F32, tag="ps")
                        for ei in range(EC):
                            nc.tensor.matmul(
                                ps[:],
                                lhsT=(wk_sb[:, ei, eo * P : (eo + 1) * P]),
                                rhs=(xblk[:, ei, :]),
                                start=(ei == 0),
                                stop=(ei == EC - 1),
                            )
                        nc.scalar.activation(
                            kt[:, eo, kvb * LB : (kvb + 1) * LB],
                            ps[:],
                            AF.Identity,
                            bias=bk_sb[:, eo : eo + 1],
                        )
                    # v = (xkvT.T @ WvT) + bv -> SBUF resident, [kv, E] layout
                    for k2 in range(LB // P):
                        kvc = kvb * (LB // P) + k2
                        for eo2 in range(eo2_n):
                            ps = pp1.tile([P, 512], F32, tag="ps")
                            for ei in range(EC):
                                nc.tensor.matmul(
                                    ps[:],
                                    lhsT=(xblk[:, ei, k2 * P : (k2 + 1) * P]),
                                    rhs=(
                                        wv_sb[:, ei, eo2 * 512 : (eo2 + 1) * 512]
                                    ),
                                    start=(ei == 0),
                                    stop=(ei == EC - 1),
                                )
                            nc.vector.tensor_tensor(
                                vv[:, kvc, eo2 * 512 : (eo2 + 1) * 512],
                                ps[:],
                                bvr_sb[:, eo2 * 512 : (eo2 + 1) * 512],
                                ALU.add,
                            )

            # wo + bo + first two mask blocks stream in behind phase 1.
            wo_sb = _load_w_halves(nc, wpool, wo_v, nc.sync)
            nc.sync.dma_start(out=bor_sb[:], in_=bor)
            mt = {}
            for i in range(2):
                mtile = maskp.tile([P, kvl // P, QB], BF16, tag="mask")
                nc.sync.dma_start(out=mtile[:], in_=maskb[i])
                mt[i] = mtile

            # ---------------- Phase 2: attention + output ----------------
            with (
                tc.tile_pool(name="p2pt", bufs=1) as ptp,
                tc.tile_pool(name="p2o", bufs=1) as osp,
                tc.tile_pool(name="p2acc", bufs=1) as accp,
                tc.tile_pool(name="p2small", bufs=2) as smallp,
                tc.tile_pool(name="p2out", bufs=2) as outp,
                tc.tile_pool(name="p2ps_s", bufs=2, space="PSUM") as pss,
                tc.tile_pool(name="p2ps_rs", bufs=2, space="PSUM") as psr,
                tc.tile_pool(name="p2ps_o", bufs=2, space="PSUM") as pso,
                tc.tile_pool(name="p2ps_f", bufs=2, space="PSUM") as psf,
            ):
                for iqb in range(nqb):
                    pt = ptp.tile([P, kc, QB], BF16, tag="pt")
                    acc = accp.tile([P, QB], F32R, tag="acc")
                    for c in range(kc):
                        ps = pss.tile([P, QB], F32, tag="s")
                        for e in range(EC):
                            nc.tensor.matmul(
                                ps[:],
                                lhsT=(kt[:, e, c * P : (c + 1) * P]),
                                rhs=(qt[:, e, iqb * QB : (iqb + 1) * QB]),
                                start=(e == 0),
                                stop=(e == EC - 1),
                            )
                        nc.scalar.activation(
                            pt[:, c, :], ps[:], AF.Exp, scale=SCALE
                        )
                        nc.vector.tensor_tensor(
                            pt[:, c, :], pt[:, c, :], mt[iqb][:, c, :], ALU.mult
                        )
                        # running kv-chunk sum for the softmax denominator
                        if c == 0:
                            nc.vector.tensor_scalar_add(
                                acc[:], pt[:, 0, :], 0.0
                            )
                        else:
                            nc.vector.tensor_tensor(
                                acc[:], acc[:], pt[:, c, :], ALU.add
                            )
                    # prefetch the mask two blocks out (slot freed by the
                    # multiplies above)
                    if iqb + 2 < nqb:
                        mtile = maskp.tile([P, kvl // P, QB], BF16, tag="mask")
                        nc.scalar.dma_start(out=mtile[:], in_=maskb[iqb + 2])
                        mt[iqb + 2] = mtile

                    osb = osp.tile([P, EC, QB], BF16, tag="o")

                    def av_group(m):
                        po = pso.tile([P, QB], F32, tag="o")
                        for c in range(kc):
                            nc.tensor.matmul(
                                po[:],
                                lhsT=(vv[:, c, m * P : (m + 1) * P]),
                                rhs=(pt[:, c, :]),
                                start=(c == 0),
                                stop=(c == kc - 1),
                            )
                        nc.scalar.activation(osb[:, m, :], po[:], AF.Copy)

                    av_group(0)
                    # row sums: one tiny matmul per 128-q chunk against the
                    # DVE-accumulated acc (cheap LDWEIGHTS vs 64 pt-chunks)
                    recip = smallp.tile([P, qq_n], F32, tag="recip")
                    for qq in range(qq_n):
                        rs = psr.tile([P, 4], F32, tag="rs")
                        nc.tensor.matmul(
                            rs[:],
                            lhsT=(acc[:, qq * P : (qq + 1) * P]),
                            rhs=(ones[:]),
                            start=True,
                            stop=True,
                        )
                        nc.vector.reciprocal(recip[:, qq : qq + 1], rs[:, 0:1])
                    for m in range(1, EC):
                        av_group(m)

                    for eo2 in range(eo2_n):
                        for qq in range(qq_n):
                            pf = psf.tile([P, 512], F32, tag="f")
                            for m in range(EC):
                                nc.tensor.matmul(
                                    pf[:],
                                    lhsT=(osb[:, m, qq * P : (qq + 1) * P]),
                                    rhs=(
                                        wo_sb[:, m, eo2 * 512 : (eo2 + 1) * 512]
                                    ),
                                    start=(m == 0),
                                    stop=(m == EC - 1),
                                )
                            ot = outp.tile([P, 512], F32, tag="out")
                            nc.vector.scalar_tensor_tensor(
                                ot[:],
                                pf[:],
                                recip[:, qq : qq + 1],
                                bor_sb[:, eo2 * 512 : (eo2 + 1) * 512],
                                ALU.mult,
                                ALU.add,
                            )
                            nc.sync.dma_start(
                                out=y[
                                    iqb * QB + qq * P : iqb * QB + (qq + 1) * P,
                                    eo2 * 512 : (eo2 + 1) * 512,
                                ],
                                in_=ot[:],
                            )

    _split_multi_waits(nc)
    return nc


_NC_CACHE = {}


def _get_nc(ql=QL, kvl=KVL):
    key = (ql, kvl)
    if key not in _NC_CACHE:
        _NC_CACHE[key] = build_nc(ql=ql, kvl=kvl)
    return _NC_CACHE[key]


def _bf16(a):
    import ml_dtypes

    return np.ascontiguousarray(a).astype(ml_dtypes.bfloat16)


def _host_prep(query, key_value, attention_mask, Wq, bq, Wk, bk, Wv, bv, Wo, bo):
    """Build the 8 per-core input maps (numpy only)."""
    b, ql, e = query.shape
    kvl = key_value.shape[1]
    kc, nqb = kvl // P, ql // QB

    f32 = np.float32
    shared = {
        "wqT": _bf16(Wq.T),
        "wkT": _bf16(Wk.T),
        "wvT": _bf16(Wv.T),
        "woT": _bf16(Wo.T),
        "bq_pp": np.ascontiguousarray(bq.reshape(EC, P).T, dtype=f32),
        "bk_pp": np.ascontiguousarray(bk.reshape(EC, P).T, dtype=f32),
        "bv_rep": np.ascontiguousarray(np.broadcast_to(bv, (P, e)), dtype=f32),
        "bo_rep": np.ascontiguousarray(np.broadcast_to(bo, (P, e)), dtype=f32),
        "ones": np.ones((P, 4), dtype=f32),
    }
    in_maps = []
    for i in range(b):
        m = attention_mask[i].T.astype(f32)  # [kv, q]
        mblk = _bf16(m.reshape(kc, P, nqb, QB).transpose(2, 1, 0, 3))
        xqb = _bf16(
            query[i].T.reshape(EC, P, ql // LB, LB).transpose(2, 1, 0, 3)
        )
        xkvb = _bf16(
            key_value[i].T.reshape(EC, P, kvl // LB, LB).transpose(2, 1, 0, 3)
        )
        in_maps.append(
            dict(shared, xq_blk=xqb, xkv_blk=xkvb, maskblk=mblk)
        )
    return in_maps


def run(inputs, trace=False):
    """Run on 8 cores; returns (output [B, QL, E], BassKernelResults)."""
    nc = _get_nc()
    in_maps = _host_prep(**inputs)
    res = run_bass_kernel_spmd(
        nc, in_maps, list(range(8)), trace=trace, trace_cores=[0]
    )
    out = np.stack([res.results[i]["y"] for i in range(8)], axis=0)
    return out, res


def kernel(**inputs):
    out, _ = run(inputs, trace=False)
    return out


# revision 12
# speedup vs baseline: 1.3264x; 1.0061x over previous
"""Trainium2 Bass kernel for nn_CrossAttention (B=8, QL=KVL=2048, E=1024).

Sharding: data-parallel over batch — batch b runs on NeuronCore b.

Per-core dataflow, all-bf16 operands (fp32 PSUM accumulation):
  P1: qT, kT -> SBUF-resident bf16 [feat-part, seq]; v -> SBUF-resident
      bf16 [seq-part, feat], K and V computed in one pass over xkv.
  P2, per 512-wide q block: sT = kT.T@qT (PSUM f32), pT = exp(sT/32)
      (bf16) * mask, row sums accumulated on DVE then one tiny matmul per
      128-q chunk, oT = v.T@pT, y = (oT.T@WoT)*recip + bo.
  No DRAM bounce: everything stays resident; the only phase-2 DMA is the
  (prefetched) mask stream in and y out.  Two HWDGE queues (sync + ACT)
  split weight/x loads so the PE primes in ~3us.
"""

import os
import sys

import numpy as np

for _p in ("/opt/trn_rl_repo", "/opt/pypackages"):
    if _p not in sys.path and os.path.isdir(_p):
        sys.path.append(_p)

import concourse.bass as bass
import concourse.mybir as mybir
import concourse.tile as tile
from concourse.bass_utils import run_bass_kernel_spmd
from concourse.vector_clock import ScopedClock

F32 = mybir.dt.float32
F32R = mybir.dt.float32r
BF16 = mybir.dt.bfloat16
AF = mybir.ActivationFunctionType
ALU = mybir.AluOpType


def _ensure_ntff_hook():
    """The agent image's antenv lacks axon_hooks, so the boot-time NTFF
    profile hook registration silently degraded. Recreate the module and
    register the ctypes-based hook against libaxon_pjrt.so so trace=True
    runs produce per-core NTFF profiles (HW exec time)."""
    try:
        from antenv.axon_hooks import get_axon_ntff_profile_hook  # noqa: F401

        return
    except ImportError:
        pass
    import contextlib
    import ctypes
    import types

    import antenv

    mod = types.ModuleType("antenv.axon_hooks")
    mod._hook = None

    def set_axon_ntff_profile_hook(h):
        mod._hook = h

    def get_axon_ntff_profile_hook():
        return mod._hook

    mod.set_axon_ntff_profile_hook = set_axon_ntff_profile_hook
    mod.get_axon_ntff_profile_hook = get_axon_ntff_profile_hook
    sys.modules["antenv.axon_hooks"] = mod
    antenv.axon_hooks = mod

    so_path = "/opt/axon/libaxon_pjrt.so"
    if not os.path.exists(so_path):
        return
    lib = ctypes.CDLL(so_path)
    if not hasattr(lib, "axon_start_nrt_profile"):
        return
    lib.axon_start_nrt_profile.argtypes = [
        ctypes.POINTER(ctypes.c_int64),
        ctypes.c_size_t,
    ]
    lib.axon_start_nrt_profile.restype = ctypes.c_int64
    lib.axon_stop_nrt_profile.argtypes = [ctypes.c_char_p]
    lib.axon_stop_nrt_profile.restype = ctypes.c_int64

    @contextlib.contextmanager
    def _hook(output_dir, device_ids):
        import jax

        jax.devices()
        if device_ids:
            ids = (ctypes.c_int64 * len(device_ids))(*device_ids)
            rc = lib.axon_start_nrt_profile(ids, len(device_ids))
        else:
            rc = lib.axon_start_nrt_profile(None, 0)
        if rc != 0:
            raise RuntimeError(f"axon_start_nrt_profile rc={rc}")
        try:
            yield
        finally:
            n = lib.axon_stop_nrt_profile(str(output_dir).encode())
            print(f"ntff profile: {n} file(s) written to {output_dir}")

    set_axon_ntff_profile_hook(_hook)


_ensure_ntff_hook()

B, QL, KVL, E = 8, 2048, 2048, 1024
P = 128
EC = E // P          # 8 feature chunks
SCALE = 1.0 / 32.0   # 1/sqrt(E)
QB = 512             # q block (moving-operand width) in phase 2
LB = 512             # x block in phase 1


class _TC(tile.TileContext):
    """TileContext whose final drain never carries >1 sync wait.

    The walrus build in this container rejects instructions with more than
    one sync-wait command; spread the drain's waits across single-wait NOPs.
    """

    def _drain_and_barrier(self, tick_clock, wait_clock):
        nc = self.nc
        probe = nc.sync.nop(nofuse=True, hint="drain_wait_probe")
        wait_clock.add_sem_waits(
            probe.ins, ScopedClock({None: tick_clock.global_clock})
        )
        si = probe.ins.sync_info
        waits = list(si.on_wait) if si is not None else []
        if len(waits) > 1:
            probe.ins.sync_info = mybir.SyncInfo(
                on_wait=waits[:1], on_update=list(si.on_update)
            )
            for w in waits[1:]:
                extra = nc.sync.nop(nofuse=True, hint="drain_wait_spill")
                extra.ins.sync_info = mybir.SyncInfo(on_wait=[w], on_update=[])
        nc.sync.drain()
        nc.all_engine_barrier()
        assert self.sems is not None
        popped = nc._tile_sem_poison_stack.pop()
        assert popped is self._sem_poison
        nc.clear_and_free_semaphores(list(self.sems.allocated().values()))
        nc.all_engine_barrier()


def _split_multi_waits(nc):
    """Walrus here allows only one sync-wait per instruction; hoist extras
    onto same-engine NOPs inserted immediately before."""
    idx = 0
    for fn in nc.m.functions:
        for blk in fn.blocks:
            out = []
            changed = False
            for inst in blk.instructions:
                si = inst.sync_info
                if si is not None and len(si.on_wait) > 1:
                    changed = True
                    waits = list(si.on_wait)
                    for w in waits[:-1]:
                        nop = mybir.InstNoOp(name=f"I-waitsplit-{idx}")
                        idx += 1
                        nop.engine = inst.engine
                        nop.sync_info = mybir.SyncInfo(on_wait=[w], on_update=[])
                        out.append(nop)
                    inst.sync_info = mybir.SyncInfo(
                        on_wait=[waits[-1]], on_update=list(si.on_update)
                    )
                out.append(inst)
            if changed:
                blk.instructions = out


class _WParts:
    """N [P, EC, w] part-tiles presented as one [P, EC, N*w] tensor.

    Each o-slice handed to the PE must stay inside one part.
    """

    def __init__(self, parts, width):
        self._p = parts
        self._w = width

    def __getitem__(self, key):
        p, ec, o = key
        if isinstance(o, slice):
            start, stop = o.start or 0, o.stop
            i = start // self._w
            assert stop <= (i + 1) * self._w
            return self._p[i][p, ec, start - i * self._w : stop - i * self._w]
        raise TypeError(o)


def _load_w_parts(nc, wpool, w_b, eng, nparts, tagpfx):
    """Load a host-blocked weight [P, nparts, EC, width] as nparts tiles.

    Host blocking makes each part contiguous per partition, so the DMA
    runs with large descriptors, and the first matmul only waits for the
    first part rather than the whole matrix.
    """
    width = 1024 // nparts
    parts = []
    for i in range(nparts):
        t = wpool.tile([P, EC, width], BF16, tag=f"{tagpfx}{i}")
        eng.dma_start(out=t[:], in_=w_b[:, i])
        parts.append(t)
    return _WParts(parts, width)


def build_nc(ql=QL, kvl=KVL):
    """Build the single-core Bass program (same program runs on all 8 cores)."""
    kc = kvl // P        # kv chunks of 128
    nqb = ql // QB       # q blocks in phase 2
    qq_n = QB // P       # 128-row subblocks per q block
    eo2_n = E // 512

    nc = bass.Bass("TRN2", target_bir_lowering=False, debug=False)

    # activations arrive host-blocked: [block, partition, e-chunk, block-col]
    xq = nc.dram_tensor(
        "xq_blk", [ql // LB, P, EC, LB], BF16, kind="ExternalInput"
    ).ap()
    xkv = nc.dram_tensor(
        "xkv_blk", [kvl // LB, P, EC, LB], BF16, kind="ExternalInput"
    ).ap()
    maskb = nc.dram_tensor(
        "maskblk", [nqb, P, kc, QB], BF16, kind="ExternalInput"
    ).ap()
    # weights arrive host-blocked [p, part, ec, width] so each part is a
    # contiguous per-partition DMA and the first matmul waits only for the
    # first 512KB part
    wq = nc.dram_tensor("wq_blk", [P, 4, EC, 256], BF16, kind="ExternalInput").ap()
    wk = nc.dram_tensor("wk_blk", [P, 4, EC, 256], BF16, kind="ExternalInput").ap()
    wv = nc.dram_tensor("wv_blk", [P, 2, EC, 512], BF16, kind="ExternalInput").ap()
    wo = nc.dram_tensor("wo_blk", [P, 2, EC, 512], BF16, kind="ExternalInput").ap()
    bq = nc.dram_tensor("bq_pp", [P, EC], F32, kind="ExternalInput").ap()
    bk = nc.dram_tensor("bk_pp", [P, EC], F32, kind="ExternalInput").ap()
    bvr = nc.dram_tensor("bv_rep", [P, E], F32, kind="ExternalInput").ap()
    bor = nc.dram_tensor("bo_rep", [P, E], F32, kind="ExternalInput").ap()
    ones_in = nc.dram_tensor("ones", [P, 4], F32R, kind="ExternalInput").ap()
    y = nc.dram_tensor("y", [ql, E], F32, kind="ExternalOutput").ap()

    with _TC(nc) as tc:
        with (
            tc.tile_pool(name="persist", bufs=1) as persist,
            tc.tile_pool(name="consts", bufs=1) as consts,
            tc.tile_pool(name="wvo", bufs=1) as wvo,
            tc.tile_pool(name="maskp", bufs=1) as maskp,
        ):
            kt = persist.tile([P, EC, kvl], BF16, tag="kt")
            qt = persist.tile([P, EC, ql], BF16, tag="qt")
            vv = persist.tile([P, kc, E], BF16, tag="vv")

            bq_sb = consts.tile([P, EC], F32, tag="bq")
            bk_sb = consts.tile([P, EC], F32, tag="bk")
            bvr_sb = consts.tile([P, E], F32, tag="bvr")
            bor_sb = consts.tile([P, E], F32, tag="bor")
            ones = consts.tile([P, 4], F32R, tag="ones")

            # ---------------- Phase 1: projections ----------------
            # sync queue: weights + biases (phase-1 critical path);
            # scalar(ACT) queue: x blocks.  The two HWDGE rings drain in
            # parallel so the first matmul starts after ~0.5MB + 1MB.
            with (
                tc.tile_pool(name="p1x", bufs=2) as xpool,
                tc.tile_pool(name="wqk", bufs=2) as wqk,
                tc.tile_pool(name="p1ps", bufs=3, space="PSUM") as pp1,
            ):
                xblk = xpool.tile([P, EC, LB], BF16, tag="x")
                nc.scalar.dma_start(out=xblk[:], in_=xq[0])
                wq_sb = _load_w_parts(nc, wqk, wq, nc.sync, 4, "q")
                nc.sync.dma_start(out=bq_sb[:], in_=bq)
                nc.sync.dma_start(out=bk_sb[:], in_=bk)

                # qT = (WqT.T @ xqT) + bq -> SBUF resident bf16
                for qlb in range(ql // LB):
                    if qlb > 0:
                        xblk = xpool.tile([P, EC, LB], BF16, tag="x")
                        nc.scalar.dma_start(out=xblk[:], in_=xq[qlb])
                    for eo in range(EC):
                        ps = pp1.tile([P, LB], F32, tag="ps")
                        for ei in range(EC):
                            nc.tensor.matmul(
                                ps[:],
                                lhsT=(wq_sb[:, ei, eo * P : (eo + 1) * P]),
                                rhs=(xblk[:, ei, :]),
                                start=(ei == 0),
                                stop=(ei == EC - 1),
                            )
                        nc.scalar.activation(
                            qt[:, eo, qlb * LB : (qlb + 1) * LB],
                            ps[:],
                            AF.Identity,
                            bias=bq_sb[:, eo : eo + 1],
                        )

                # kT and v in one pass over xkv.
                xblk = xpool.tile([P, EC, LB], BF16, tag="x")
                nc.scalar.dma_start(out=xblk[:], in_=xkv[0])
                wk_sb = _load_w_parts(nc, wqk, wk, nc.sync, 4, "q")
                wv_sb = _load_w_parts(nc, wvo, wv, nc.sync, 2, "vh")
                nc.sync.dma_start(out=bvr_sb[:], in_=bvr)
                nc.sync.dma_start(out=ones[:], in_=ones_in)
                # wo + bo + the first mask block stream in behind phase 1
                wo_sb = _load_w_parts(nc, wvo, wo, nc.sync, 2, "oh")
                nc.sync.dma_start(out=bor_sb[:], in_=bor)
                mt = {}
                mtile = maskp.tile([P, kvl // P, QB], BF16, tag="mask")
                nc.sync.dma_start(out=mtile[:], in_=maskb[0])
                mt[0] = mtile
                for kvb in range(kvl // LB):
                    if kvb > 0:
                        xblk = xpool.tile([P, EC, LB], BF16, tag="x")
                        nc.scalar.dma_start(out=xblk[:], in_=xkv[kvb])
                    # kT = (WkT.T @ xkvT) + bk -> SBUF resident bf16
                    for eo in range(EC):
                        ps = pp1.tile([P, LB], F32, tag="ps")
                        for ei in range(EC):
                            nc.tensor.matmul(
                                ps[:],
                                lhsT=(wk_sb[:, ei, eo * P : (eo + 1) * P]),
                                rhs=(xblk[:, ei, :]),
                                start=(ei == 0),
                                stop=(ei == EC - 1),
                            )
                        nc.scalar.activation(
                            kt[:, eo, kvb * LB : (kvb + 1) * LB],
                            ps[:],
                            AF.Identity,
                            bias=bk_sb[:, eo : eo + 1],
                        )
                    # v = (xkvT.T @ WvT) + bv -> SBUF resident, [kv, E] layout
                    for k2 in range(LB // P):
                        kvc = kvb * (LB // P) + k2
                        for eo2 in range(eo2_n):
                            ps = pp1.tile([P, 512], F32, tag="ps")
                            for ei in range(EC):
                                nc.tensor.matmul(
                                    ps[:],
                                    lhsT=(xblk[:, ei, k2 * P : (k2 + 1) * P]),
                                    rhs=(
                                        wv_sb[:, ei, eo2 * 512 : (eo2 + 1) * 512]
                                    ),
                                    start=(ei == 0),
                                    stop=(ei == EC - 1),
                                )
                            nc.vector.tensor_tensor(
                                vv[:, kvc, eo2 * 512 : (eo2 + 1) * 512],
                                ps[:],
                                bvr_sb[:, eo2 * 512 : (eo2 + 1) * 512],
                                ALU.add,
                            )

            # ---------------- Phase 2: attention + output ----------------
            with (
                tc.tile_pool(name="p2pt", bufs=1) as ptp,
                tc.tile_pool(name="p2o", bufs=1) as osp,
                tc.tile_pool(name="p2acc", bufs=1) as accp,
                tc.tile_pool(name="p2small", bufs=2) as smallp,
                tc.tile_pool(name="p2out", bufs=2) as outp,
                tc.tile_pool(name="p2ps_s", bufs=2, space="PSUM") as pss,
                tc.tile_pool(name="p2ps_rs", bufs=2, space="PSUM") as psr,
                tc.tile_pool(name="p2ps_o", bufs=2, space="PSUM") as pso,
                tc.tile_pool(name="p2ps_f", bufs=2, space="PSUM") as psf,
            ):
                for iqb in range(nqb):
                    pt = ptp.tile([P, kc, QB], BF16, tag="pt")
                    acc = accp.tile([P, QB], F32R, tag="acc")
                    for c in range(kc):
                        ps = pss.tile([P, QB], F32, tag="s")
                        for e in range(EC):
                            nc.tensor.matmul(
                                ps[:],
                                lhsT=(kt[:, e, c * P : (c + 1) * P]),
                                rhs=(qt[:, e, iqb * QB : (iqb + 1) * QB]),
                                start=(e == 0),
                                stop=(e == EC - 1),
                            )
                        nc.scalar.activation(
                            pt[:, c, :], ps[:], AF.Exp, scale=SCALE
                        )
                        nc.vector.tensor_tensor(
                            pt[:, c, :], pt[:, c, :], mt[iqb][:, c, :], ALU.mult
                        )
                        # running kv-chunk sum for the softmax denominator
                        if c == 0:
                            nc.vector.tensor_scalar_add(
                                acc[:], pt[:, 0, :], 0.0
                            )
                        else:
                            nc.vector.tensor_tensor(
                                acc[:], acc[:], pt[:, c, :], ALU.add
                            )
                    # prefetch the next mask block (slot freed by the
                    # multiplies above; needed only after this block's
                    # AV + output projection)
                    if iqb + 1 < nqb:
                        mtile = maskp.tile([P, kvl // P, QB], BF16, tag="mask")
                        nc.scalar.dma_start(out=mtile[:], in_=maskb[iqb + 1])
                        mt[iqb + 1] = mtile

                    osb = osp.tile([P, EC, QB], BF16, tag="o")

                    def av_group(m):
                        po = pso.tile([P, QB], F32, tag="o")
                        for c in range(kc):
                            nc.tensor.matmul(
                                po[:],
                                lhsT=(vv[:, c, m * P : (m + 1) * P]),
                                rhs=(pt[:, c, :]),
                                start=(c == 0),
                                stop=(c == kc - 1),
                            )
                        nc.scalar.activation(osb[:, m, :], po[:], AF.Copy)

                    av_group(0)
                    # row sums: one tiny matmul per 128-q chunk against the
                    # DVE-accumulated acc (cheap LDWEIGHTS vs 64 pt-chunks)
                    recip = smallp.tile([P, qq_n], F32, tag="recip")
                    for qq in range(qq_n):
                        rs = psr.tile([P, 4], F32, tag="rs")
                        nc.tensor.matmul(
                            rs[:],
                            lhsT=(acc[:, qq * P : (qq + 1) * P]),
                            rhs=(ones[:]),
                            start=True,
                            stop=True,
                        )
                        nc.vector.reciprocal(recip[:, qq : qq + 1], rs[:, 0:1])
                    for m in range(1, EC):
                        av_group(m)

                    for eo2 in range(eo2_n):
                        for qq in range(qq_n):
                            pf = psf.tile([P, 512], F32, tag="f")
                            for m in range(EC):
                                nc.tensor.matmul(
                                    pf[:],
                                    lhsT=(osb[:, m, qq * P : (qq + 1) * P]),
                                    rhs=(
                                        wo_sb[:, m, eo2 * 512 : (eo2 + 1) * 512]
                                    ),
                                    start=(m == 0),
                                    stop=(m == EC - 1),
                                )
                            ot = outp.tile([P, 512], F32, tag="out")
                            nc.vector.scalar_tensor_tensor(
                                ot[:],
                                pf[:],
                                recip[:, qq : qq + 1],
                                bor_sb[:, eo2 * 512 : (eo2 + 1) * 512],
                                ALU.mult,
                                ALU.add,
                            )
                            nc.sync.dma_start(
                                out=y[
                                    iqb * QB + qq * P : iqb * QB + (qq + 1) * P,
                                    eo2 * 512 : (eo2 + 1) * 512,
                                ],
                                in_=ot[:],
                            )

    _split_multi_waits(nc)
    return nc


_NC_CACHE = {}


def _get_nc(ql=QL, kvl=KVL):
    key = (ql, kvl)
    if key not in _NC_CACHE:
        _NC_CACHE[key] = build_nc(ql=ql, kvl=kvl)
    return _NC_CACHE[key]


def _bf16(a):
    import ml_dtypes

    return np.ascontiguousarray(a).astype(ml_dtypes.bfloat16)


def _host_prep(query, key_value, attention_mask, Wq, bq, Wk, bk, Wv, bv, Wo, bo):
    """Build the 8 per-core input maps (numpy only)."""
    b, ql, e = query.shape
    kvl = key_value.shape[1]
    kc, nqb = kvl // P, ql // QB

    f32 = np.float32

    def wblk(W, nparts):
        # [P, nparts, EC, width]: part-contiguous per partition
        width = E // nparts
        return _bf16(W.T.reshape(EC, P, nparts, width).transpose(1, 2, 0, 3))

    shared = {
        "wq_blk": wblk(Wq, 4),
        "wk_blk": wblk(Wk, 4),
        "wv_blk": wblk(Wv, 2),
        "wo_blk": wblk(Wo, 2),
        "bq_pp": np.ascontiguousarray(bq.reshape(EC, P).T, dtype=f32),
        "bk_pp": np.ascontiguousarray(bk.reshape(EC, P).T, dtype=f32),
        "bv_rep": np.ascontiguousarray(np.broadcast_to(bv, (P, e)), dtype=f32),
        "bo_rep": np.ascontiguousarray(np.broadcast_to(bo, (P, e)), dtype=f32),
        "ones": np.ones((P, 4), dtype=f32),
    }
    in_maps = []
    for i in range(b):
        m = attention_mask[i].T.astype(f32)  # [kv, q]
        mblk = _bf16(m.reshape(kc, P, nqb, QB).transpose(2, 1, 0, 3))
        xqb = _bf16(
            query[i].T.reshape(EC, P, ql // LB, LB).transpose(2, 1, 0, 3)
        )
        xkvb = _bf16(
            key_value[i].T.reshape(EC, P, kvl // LB, LB).transpose(2, 1, 0, 3)
        )
        in_maps.append(
            dict(shared, xq_blk=xqb, xkv_blk=xkvb, maskblk=mblk)
        )
    return in_maps


def run(inputs, trace=False):
    """Run on 8 cores; returns (output [B, QL, E], BassKernelResults)."""
    nc = _get_nc()
    in_maps = _host_prep(**inputs)
    res = run_bass_kernel_spmd(
        nc, in_maps, list(range(8)), trace=trace, trace_cores=[0]
    )
    out = np.stack([res.results[i]["y"] for i in range(8)], axis=0)
    return out, res


def kernel(**inputs):
    out, _ = run(inputs, trace=False)
    return out
